# revision 44
# baseline (speedup 1.0000x reference)
"""Trainium2 Bass kernel for nn_ALAttention (sparse local attention).

Sharding: 64 image rows split across 8 cores (8 query rows each). All 33
attention targets of a query in row r lie within rows r-4..r+4, so each core
works on a host-sliced 16-row halo slab of x (virtually centered, zero-padded
at borders -> identical SPMD graph; padded keys are masked out). Per core:
QKV GEMM (bf16, fused bias, q-scale folded into host-prescaled bias), masked
dense local attention in S^T=[keys,q] layout (host-built mask from attn_idx,
exp -> mask-mult -> V_aug matmul whose ones-column yields the softmax
denominator), normalize via fast-reciprocal + partition-broadcast, proj GEMM.
Key chunks 0 and 7 of the 1024-key slab are only needed by the first/last
query row-pair (host-asserted), so their score/exp/AV work runs at 1/4 width.
No inter-core communication.
"""
import os
import sys
import types

sys.path.insert(0, "/opt/trn_rl_repo")

import numpy as np
import ml_dtypes

from concourse import bacc, tile, mybir
from concourse import bass_utils
from concourse import masks as bass_masks
from concourse.bass_utils import run_bass_kernel_spmd

F32 = mybir.dt.float32
BF16 = mybir.dt.bfloat16
AF = mybir.ActivationFunctionType
ALU = mybir.AluOpType

B = 2
C = 384
HH = WW = 64
HEADS = 6
NCORES = 8
ROWS = 8
SLAB = 16
SCOLS = SLAB * WW      # 1024 slab key positions
QCOLS = ROWS * WW      # 512 queries per core
NKC = SCOLS // 128     # 8 key chunks
SCALE = float(64) ** -0.5
# full-width key chunks (1..6) run for all 512 queries; chunks 0 and 7 are
# banded (only the first/last query row-pair needs them, host-asserted)
FULL_CHUNKS = (1, 2, 3, 4, 5, 6)
MASKW = 6 * 512 + 256

LAST_EXEC_NS = None
LAST_TRACE = None
_NC_CACHE = {}


def _register_ntff_hook():
    if "antenv.axon_hooks" in sys.modules:
        return
    try:
        from trn_agent_boot.trn_boot import _ntff_profile_via_ctypes
        hook = _ntff_profile_via_ctypes("/opt/axon/libaxon_pjrt.so")
    except Exception:
        hook = None
    mod = types.ModuleType("antenv.axon_hooks")
    mod.get_axon_ntff_profile_hook = lambda: hook
    mod.set_axon_ntff_profile_hook = lambda h: None
    sys.modules["antenv.axon_hooks"] = mod
    bass_utils.upload_artifacts = lambda tmpdir: "local://skipped"


def build_graph():
    nc = bacc.Bacc("TRN2", target_bir_lowering=False, debug=False,
                   num_devices=NCORES)

    xs_e = nc.dram_tensor("xs", [B, C, SCOLS], BF16, kind="ExternalInput").ap()
    wqkvT_e = nc.dram_tensor("wqkvT", [C, 3 * C], BF16, kind="ExternalInput").ap()
    bqkv_e = nc.dram_tensor("bqkv", [128, 9], F32, kind="ExternalInput").ap()
    wprojT_e = nc.dram_tensor("wprojT", [C, C], BF16, kind="ExternalInput").ap()
    bproj_e = nc.dram_tensor("bproj", [128, 3], F32, kind="ExternalInput").ap()
    mask_e = nc.dram_tensor("mask", [128, MASKW], BF16,
                            kind="ExternalInput").ap()
    out_e = nc.dram_tensor("out", [B, C, QCOLS], F32, kind="ExternalOutput").ap()

    with tile.TileContext(nc) as tc:
        with (
            tc.tile_pool(name="const", bufs=1) as cpool,
            tc.tile_pool(name="xin", bufs=2) as xpool,
            tc.tile_pool(name="qkv", bufs=2) as qkvpool,
            tc.tile_pool(name="vt", bufs=2) as vtpool,
            tc.tile_pool(name="esb", bufs=3) as epool,
            tc.tile_pool(name="osb", bufs=2) as opool,
            tc.tile_pool(name="sc", bufs=3) as scpool,
            tc.tile_pool(name="psA", bufs=2, space="PSUM") as psA,
            tc.tile_pool(name="psB", bufs=4, space="PSUM") as psB,
        ):
            # interleave x/w chunk DMAs so the first matmul is gated only by
            # its own chunks; masks go via the gpsimd SWDGE queue in parallel
            x_sb0 = xpool.tile([128, 3, SCOLS], BF16, tag="x", name="x_sb0")
            w0_sb = cpool.tile([128, 3, 128], BF16, tag="wqkv0")
            w_sb = cpool.tile([128, 3, 3 * C], BF16, tag="wqkv")
            qs = [nc.sync, nc.scalar, nc.gpsimd]
            for k in range(3):
                qs[k].dma_start(x_sb0[:, k, :], xs_e[0, 128 * k:128 * (k + 1), :])
                qs[(k + 1) % 3].dma_start(w0_sb[:, k, :],
                                          wqkvT_e[128 * k:128 * (k + 1), 0:128])
            for k in range(3):
                qs[k].dma_start(w_sb[:, k, :],
                                wqkvT_e[128 * k:128 * (k + 1), :])
            bq_sb = cpool.tile([128, 9], F32, tag="bqkv")
            nc.sync.dma_start(bq_sb[:], bqkv_e[:])
            bp_sb = cpool.tile([128, 3], F32, tag="bproj")
            nc.sync.dma_start(bp_sb[:], bproj_e[:])
            ident = cpool.tile([128, 128], BF16, tag="ident")
            bass_masks.make_identity(nc, ident[:])
            # pre-warm the scalar engine's EXP table
            warm_sb = cpool.tile([1, 1], F32, tag="warm")
            nc.scalar.activation(warm_sb[:], ident[0:1, 0:1], AF.Exp)
            mask_sb = cpool.tile([128, MASKW], BF16, tag="mask")
            nc.gpsimd.dma_start(mask_sb[:], mask_e[:])
            wp_sb = cpool.tile([128, 3, C], BF16, tag="wproj")
            for k in range(3):
                nc.scalar.dma_start(wp_sb[:, k, :],
                                    wprojT_e[128 * k:128 * (k + 1), :])

            # ---- QKV GEMMs for BOTH batches first: one long dense PE run
            # that warms the HAM clock gate and stays ahead of attention ----
            qkv_mb = []
            for b in range(B):
                if b == 0:
                    x_sb = x_sb0
                else:
                    x_sb = xpool.tile([128, 3, SCOLS], BF16, tag="x",
                                      name="x_sb1")
                    for k in range(3):
                        nc.gpsimd.dma_start(x_sb[:, k, :],
                                            xs_e[b, 128 * k:128 * (k + 1), :])

                qkv_m = [qkvpool.tile([128, SCOLS], BF16, tag=f"qkv{m}",
                                      name=f"qkv{m}_{b}") for m in range(9)]
                qkv_mb.append(qkv_m)
                for m in (0, 3, 6, 1, 4, 7, 2, 5, 8):
                    is_q = m < 3
                    ps = psA.tile([128, 1024], F32, tag="mm")
                    if is_q:
                        for k in range(3):
                            nc.tensor.matmul(
                                ps[:, 0:512],
                                w0_sb[:, k, :] if m == 0
                                else w_sb[:, k, 128 * m:128 * (m + 1)],
                                x_sb[:, k, 256:768],
                                start=(k == 0), stop=(k == 2))
                        nc.scalar.activation(
                            qkv_m[m][:, 256:768], ps[:, 0:512],
                            AF.Identity, bias=bq_sb[:, m:m + 1], scale=SCALE)
                    else:
                        for n in range(2):
                            for k in range(3):
                                nc.tensor.matmul(
                                    ps[:, 512 * n:512 * (n + 1)],
                                    w_sb[:, k, 128 * m:128 * (m + 1)],
                                    x_sb[:, k, 512 * n:512 * (n + 1)],
                                    start=(k == 0), stop=(k == 2))
                        if m % 2 == 0:
                            nc.scalar.activation(
                                qkv_m[m][:], ps[:],
                                AF.Identity, bias=bq_sb[:, m:m + 1], scale=1.0)
                        else:
                            nc.vector.tensor_scalar(
                                qkv_m[m][:], ps[:], bq_sb[:, m:m + 1], None,
                                ALU.add)

            for b in range(B):
                qkv_m = qkv_mb[b]
                # ---- V transposes for ALL heads first (dense PE run that
                # re-warms the clock gate); even/odd heads pair up in the PE
                # array via row tiling (base partitions 0/64) ----
                v_sbs = []
                for c in range(3):
                    v_t = qkv_m[6 + c]
                    v_ps = [psB.tile([128, NKC, 64], BF16, tag="acc",
                                     name=f"v_ps{c}{eo}_{b}")
                            for eo in range(2)]
                    for j in range(NKC):
                        for eo in range(2):
                            mo = 64 * eo
                            nc.tensor.transpose(
                                v_ps[eo][:, j, :],
                                v_t[mo:mo + 64, 128 * j:128 * (j + 1)],
                                ident[mo:mo + 64, mo:mo + 64])
                    for eo in range(2):
                        v_sb = vtpool.tile([128, NKC, 128], BF16,
                                           tag=f"vt{2 * c + eo}",
                                           name=f"v_sb{2 * c + eo}_{b}")
                        nc.vector.tensor_copy(v_sb[:, :, 0:64], v_ps[eo][:])
                        nc.gpsimd.memset(v_sb[:, :, 64:65], 1.0)
                        v_sbs.append(v_sb)

                # ---- attention per head-pair; st matmuls for the even/odd
                # heads co-reside in the PE array (row tiling) ----
                ocat = [opool.tile([128, QCOLS], BF16, tag=f"ocat{c}",
                                   name=f"ocat{c}_{b}") for c in range(3)]
                for c in range(3):
                    k_t = qkv_m[3 + c]
                    q_e = qkv_m[c][0:64, 256:768]
                    q_o = qkv_m[c][64:128, 256:768]
                    v_e, v_o = v_sbs[2 * c], v_sbs[2 * c + 1]

                    ot_e = psB.tile([128, QCOLS], F32, tag="acc",
                                    name=f"ot_e{c}_{b}")
                    ot_o = psB.tile([128, QCOLS], F32, tag="acc",
                                    name=f"ot_o{c}_{b}")
                    for gi, j in enumerate(FULL_CHUNKS):
                        st = psA.tile([128, 2, 512], F32, tag="mm",
                                      name=f"st{c}_{gi}_{b}")
                        # density filler: overwritten by the start=True
                        # matmul below; keeps the PE activity monitor warm
                        nc.tensor.matmul(
                            st[:, 0, 0:512], k_t[0:64, 0:128],
                            q_e, start=True, stop=True)
                        nc.tensor.matmul(
                            st[:, 0, :],
                            k_t[0:64, 128 * j:128 * (j + 1)],
                            q_e, start=True, stop=True)
                        nc.tensor.matmul(
                            st[:, 1, :],
                            k_t[64:128, 128 * j:128 * (j + 1)],
                            q_o, start=True, stop=True)
                        e_sb = epool.tile([128, 2, 512], BF16, tag="e",
                                          name=f"e{c}_{gi}_{b}")
                        nc.scalar.activation(e_sb[:], st[:], AF.Exp)
                        nc.vector.tensor_tensor(
                            e_sb[:], e_sb[:],
                            mask_sb[:, 512 * gi:512 * (gi + 1)][:, None, :]
                            .to_broadcast([128, 2, 512]),
                            ALU.mult)
                        nc.tensor.matmul(
                            ot_e[:], v_e[:, j, :], e_sb[:, 0, :],
                            start=(gi == 0), stop=False,
                            skip_group_check=True)
                        nc.tensor.matmul(
                            ot_o[:], v_o[:, j, :], e_sb[:, 1, :],
                            start=(gi == 0), stop=False,
                            skip_group_check=True)

                    # banded tail: chunk 0 -> queries 0:128, chunk 7 ->
                    # queries 384:512 (per head)
                    for eo, (q_h, v_h, ot_h) in enumerate(
                            ((q_e, v_e, ot_e), (q_o, v_o, ot_o))):
                        mo = 64 * eo
                        st = psA.tile([128, 2, 512], F32, tag="mm",
                                      name=f"stt{c}{eo}_{b}")
                        nc.tensor.matmul(st[:, 0, 0:128],
                                         k_t[mo:mo + 64, 0:128],
                                         q_h[:, 0:128], start=True, stop=True)
                        nc.tensor.matmul(st[:, 0, 128:256],
                                         k_t[mo:mo + 64, 896:1024],
                                         q_h[:, 384:512], start=True,
                                         stop=True)
                        e_sb = epool.tile([128, 2, 512], BF16, tag="e",
                                          name=f"et{c}{eo}_{b}")
                        nc.scalar.activation(e_sb[:, 0, 0:256],
                                             st[:, 0, 0:256], AF.Exp)
                        nc.vector.tensor_tensor(
                            e_sb[:, 0, 0:256], e_sb[:, 0, 0:256],
                            mask_sb[:, 3072:3328], ALU.mult)
                        nc.tensor.matmul(ot_h[:, 0:128], v_h[:, 0, :],
                                         e_sb[:, 0, 0:128], start=False,
                                         stop=False, skip_group_check=True)
                        nc.tensor.matmul(ot_h[:, 384:512], v_h[:, 7, :],
                                         e_sb[:, 0, 128:256], start=False,
                                         stop=True, skip_group_check=True)

                    for eo, ot in ((0, ot_e), (1, ot_o)):
                        mo = 64 * eo
                        srow = scpool.tile([1, QCOLS], F32, tag="srow")
                        nc.vector.tensor_copy(srow[:], ot[64:65, :])
                        rrow = scpool.tile([1, QCOLS], F32, tag="rrow")
                        nc.vector.reciprocal_approx_fast(rrow[:], srow[:])
                        rb = scpool.tile([64, QCOLS], F32, tag="rb")
                        nc.gpsimd.partition_broadcast(rb[:], rrow[:])
                        nc.vector.tensor_tensor(
                            ocat[c][mo:mo + 64, :], ot[0:64, :], rb[:],
                            ALU.mult)

                # ---- proj GEMM + bias; k-outer ----
                pps = [psB.tile([128, QCOLS], F32, tag="acc",
                                name=f"pp{m}_{b}") for m in range(3)]
                for k in range(3):
                    for m in range(3):
                        nc.tensor.matmul(
                            pps[m][:], wp_sb[:, k, 128 * m:128 * (m + 1)],
                            ocat[k][:], start=(k == 0), stop=(k == 2),
                            skip_group_check=True)
                outq = [nc.sync, nc.scalar, nc.gpsimd]
                for m in range(3):
                    o_sb = scpool.tile([128, QCOLS], F32, tag="out")
                    if m == 1:
                        nc.vector.tensor_scalar(
                            o_sb[:], pps[m][:], bp_sb[:, m:m + 1], None,
                            ALU.add)
                    else:
                        nc.scalar.activation(
                            o_sb[:], pps[m][:], AF.Identity,
                            bias=bp_sb[:, m:m + 1], scale=1.0)
                    outq[m].dma_start(out_e[b, 128 * m:128 * (m + 1), :],
                                      o_sb[:])

    nc.compile()
    return nc


def _build_inputs(x, w_qkv, b_qkv, w_proj, b_proj, attn_idx):
    bf = ml_dtypes.bfloat16
    x = np.asarray(x, np.float32)
    attn_idx = np.asarray(attn_idx)

    xp = np.zeros((B, C, HH + 8, WW), np.float32)
    xp[:, :, 4:4 + HH, :] = x
    xp = xp.astype(bf)

    wqkvT = np.ascontiguousarray(np.asarray(w_qkv, np.float32).T).astype(bf)
    wprojT = np.ascontiguousarray(np.asarray(w_proj, np.float32).T).astype(bf)

    b_adj = np.asarray(b_qkv, np.float32).copy()
    b_adj[:C] *= SCALE
    bqkv = np.ascontiguousarray(b_adj.reshape(9, 128).T)
    bproj = np.ascontiguousarray(
        np.asarray(b_proj, np.float32).reshape(3, 128).T)

    in_maps = []
    for i in range(NCORES):
        slab = np.ascontiguousarray(
            xp[:, :, 8 * i:8 * i + SLAB, :]).reshape(B, C, SCOLS)
        q0 = 8 * i * WW
        gq = np.arange(q0, q0 + QCOLS)
        aidx = attn_idx[gq].astype(np.int64)
        local = aidx - (8 * i - 4) * WW
        assert local.min() >= 0 and local.max() < SCOLS, \
            f"core {i}: attn target outside slab"
        m = np.zeros((NKC, 128, QCOLS), np.float32)
        qq = np.repeat(np.arange(QCOLS), aidx.shape[1])
        ll = local.ravel()
        m[ll // 128, ll % 128, qq] = 1.0
        # banded-tail coverage: chunk 0 only serves queries 0:128,
        # chunk 7 only queries 384:512
        assert m[0, :, 128:].sum() == 0, f"core {i}: chunk0 band violated"
        assert m[7, :, :384].sum() == 0, f"core {i}: chunk7 band violated"
        # per full chunk (shared by the head pair via a step-0 broadcast
        # AP on-device); tail packed as [m0 | m7] over the banded ranges
        packed = np.zeros((128, MASKW), np.float32)
        for g, j in enumerate(FULL_CHUNKS):
            packed[:, 512 * g:512 * (g + 1)] = m[j]
        packed[:, 3072:3200] = m[0][:, 0:128]
        packed[:, 3200:3328] = m[7][:, 384:512]
        in_maps.append({
            "xs": slab,
            "wqkvT": wqkvT,
            "bqkv": bqkv,
            "wprojT": wprojT,
            "bproj": bproj,
            "mask": np.ascontiguousarray(packed).astype(bf),
        })
    return in_maps


def kernel(x, w_qkv, b_qkv, w_proj, b_proj, attn_idx):
    global LAST_EXEC_NS, LAST_TRACE
    _register_ntff_hook()
    if "graph" not in _NC_CACHE:
        _NC_CACHE["graph"] = build_graph()
    nc = _NC_CACHE["graph"]
    in_maps = _build_inputs(x, w_qkv, b_qkv, w_proj, b_proj, attn_idx)
    trace = bool(int(os.environ.get("BASSK_TRACE", "0")))
    res = run_bass_kernel_spmd(nc, in_maps, core_ids=list(range(NCORES)),
                               trace=trace)
    LAST_EXEC_NS = res.exec_time_ns
    if res.instructions_and_trace is not None:
        LAST_TRACE = res.instructions_and_trace[1]
    out = np.empty((B, C, HH, WW), np.float32)
    for i in range(NCORES):
        o = res.results[i]["out"].reshape(B, C, ROWS, WW)
        out[:, :, 8 * i:8 * i + ROWS, :] = o
    return out
